# revision 13
# baseline (speedup 1.0000x reference)
"""Trainium2 Bass kernel: 3x3 conv (stride 1, pad 1) via shifted-matmul.

Full problem: x (32, 18, 256, 256) f32, weight (64, 18, 3, 3), bias (64,)
-> out (32, 64, 256, 256).  Data-parallel over batch: 8 cores x 4 images.

Per-core algorithm:
  - Process each image in horizontal strips of R output rows.
  - SBUF strip buffer G [54, R, 258]: three 18-channel groups, group g
    holding the padded input rows shifted by g (group g row l = padded-X
    row h0+l+g), width-padded with zero columns at 0 and 257.
  - For each PSUM tile ([128, 512] = 4 output rows x 256 cols, two 64-oc
    pixel sets on PSUM partition halves), accumulate 3 matmuls (kw = 0,1,2
    as AP column offsets) per pixel set; K=54 contracts channels x kh.
  - fp32r matmul dtype (full-rate fp32 on the PE at N=512).
  - PSUM -> SBUF copy + bias on ACT/DVE, then DMA to HBM.
"""

import re
import numpy as np

import bass_rust
import concourse.bass as bass
import concourse.mybir as mybir
from concourse.tile import TileContext


# ---------------------------------------------------------------------------
# TileContext drain patch: this walrus build rejects an InstDrain carrying
# more than ~2 sync waits ("Too many sync wait commands").  Re-emit the
# end-of-kernel global-clock waits as one nop per semaphore, then drain.
# ---------------------------------------------------------------------------
def _patched_drain_and_barrier(self, tick_clock, wait_clock):
    gc = tick_clock.global_clock
    vals = [int(s) for s in re.findall(r"\d+", repr(gc))]
    for i, v in enumerate(vals):
        if v > 0:
            c = bass_rust.VectorClock()
            c.require_at_least(i, v)
            nop = self.nc.sync.nop(nofuse=True, hint=f"drain_wait_{i}")
            wait_clock.add_sem_waits(nop.ins, bass_rust.ScopedClock({None: c}))
    self.nc.sync.drain()

    self.nc.all_engine_barrier()
    assert self.sems is not None
    popped = self.nc._tile_sem_poison_stack.pop()
    assert popped is self._sem_poison
    self.nc.clear_and_free_semaphores(list(self.sems.allocated().values()))
    self.nc.all_engine_barrier()


TileContext._drain_and_barrier = _patched_drain_and_barrier


def _split_excess_waits(nc, max_waits=1):
    """This walrus build allows very few sync waits per instruction.
    Hoist excess waits onto same-engine nops placed just before."""
    for f in nc.m.functions:
        for bb in f.blocks:
            out = []
            changed = False
            for inst in bb.instructions:
                si = inst.sync_info
                waits = list(si.on_wait) if si and si.on_wait else []
                if len(waits) > max_waits:
                    changed = True
                    extras, keep = waits[:-max_waits], waits[-max_waits:]
                    for j, w in enumerate(extras):
                        nop = mybir.InstNoOp(
                            name=f"{inst.name}_xw{j}", ins=[], outs=[]
                        )
                        nop.engine = inst.engine
                        nop.sync_info = mybir.SyncInfo(on_wait=[w], on_update=[])
                        out.append(nop)
                    inst.sync_info = mybir.SyncInfo(
                        on_wait=keep,
                        on_update=list(si.on_update) if si.on_update else [],
                    )
                out.append(inst)
            if changed:
                bb.instructions = out


# ---------------------------------------------------------------------------
# Kernel builder
# ---------------------------------------------------------------------------
F32 = mybir.dt.float32
F32R = mybir.dt.float32r


def build_conv_nc(
    n_img=4,
    H=256,
    W=256,
    R=64,
    C_IN=18,
    C_OUT=64,
    mm_dtype=F32R,
    act_frac=5,  # of 9 drain tiles, how many go to ACT (rest DVE)
):
    """Build the per-core Bass program. Returns nc."""
    assert H % R == 0 and R % 4 == 0
    Wp = W + 2
    G_P = 3 * C_IN  # 54 partitions

    nc = bass.Bass()
    x = nc.dram_tensor("x", [n_img, C_IN, H, W], mm_dtype, kind="ExternalInput")
    wT = nc.dram_tensor("wT", [G_P, 3, C_OUT], mm_dtype, kind="ExternalInput")
    bias2 = nc.dram_tensor("bias2", [2 * C_OUT, 1], F32, kind="ExternalInput")
    zeros = nc.dram_tensor("zeros", [G_P, W + 2], mm_dtype, kind="ExternalInput")
    y = nc.dram_tensor("y", [n_img, C_OUT, H, W], F32, kind="ExternalOutput")

    n_strips = H // R
    tiles_per_strip = R // 2  # each PSUM tile covers 2 output rows

    with TileContext(nc) as tc:
        with (
            tc.tile_pool(name="wpool", bufs=1) as wpool,
            tc.tile_pool(name="gpool", bufs=2) as gpool,
            tc.tile_pool(name="opool", bufs=4) as opool,
            tc.tile_pool(name="psum", bufs=8, space="PSUM") as pspool,
        ):
            wsb = wpool.tile([G_P, 3, C_OUT], mm_dtype, tag="wsb")
            bsb = wpool.tile([2 * C_OUT, 1], F32, tag="bsb")
            nc.sync.dma_start(out=wsb[:], in_=wT[:])
            nc.sync.dma_start(out=bsb[:], in_=bias2[:])

            tile_idx = 0
            for n in range(n_img):
                for s in range(n_strips):
                    h0 = s * R
                    G = gpool.tile([G_P, R, Wp], mm_dtype, tag="G")
                    # zero padding columns (0 and W+1) for all rows.
                    # (DMA, not engine memset: walrus requires non-DMA
                    # producers of fp32r matmul inputs to round to fp32r,
                    # and memset lowers as an integer write.)
                    nc.sync.dma_start(
                        out=G[:, :, 0], in_=zeros[:, 0:R]
                    )
                    nc.sync.dma_start(
                        out=G[:, :, Wp - 1], in_=zeros[:, 0:R]
                    )
                    # group g holds padded-X rows [h0+g, h0+g+R)
                    #   = x rows [h0+g-1, h0+g+R-1)
                    # Boundary rows: memset the full partition range first
                    # (base partition must be 32-aligned); the groups with
                    # valid data overwrite theirs via DMA below.
                    if s == 0:
                        nc.sync.dma_start(
                            out=G[:, 0, :], in_=zeros[:, :]
                        )
                    if s == n_strips - 1:
                        nc.sync.dma_start(
                            out=G[:, R - 1, :], in_=zeros[:, :]
                        )
                    for g in range(3):
                        r_lo = h0 + g - 1
                        r_hi = h0 + g + R - 1
                        l_lo = 0
                        if r_lo < 0:
                            l_lo = -r_lo
                            r_lo = 0
                        if r_hi > H:
                            r_hi = H
                        nc.sync.dma_start(
                            out=G[
                                g * C_IN : (g + 1) * C_IN,
                                l_lo : l_lo + (r_hi - r_lo),
                                1 : W + 1,
                            ],
                            in_=x[n, :, r_lo:r_hi, :],
                        )

                    # fp32r matmul dst must start at partition 0 (the PE
                    # uses both column halves internally), so one [64, 512]
                    # PSUM tile per 2 output rows.
                    for t in range(tiles_per_strip):
                        l = 2 * t
                        PT = pspool.tile([C_OUT, 512], F32, tag="PT")
                        for b in range(3):
                            nc.tensor.matmul(
                                PT[:],
                                wsb[:, b, :],
                                G[:, l : l + 2, b : b + W],
                                start=(b == 0),
                                stop=(b == 2),
                            )
                        OB = opool.tile([C_OUT, 512], F32, tag="OB")
                        if tile_idx % 9 < act_frac:
                            nc.scalar.activation(
                                OB[:],
                                PT[:],
                                mybir.ActivationFunctionType.Identity,
                                bias=bsb[0:C_OUT],
                            )
                        else:
                            nc.vector.tensor_scalar_add(
                                OB[:], PT[:], bsb[0:C_OUT]
                            )
                        h = h0 + l
                        nc.sync.dma_start(
                            out=y[n, :, h : h + 2, :], in_=OB[:]
                        )
                        tile_idx += 1
    return nc


# ---------------------------------------------------------------------------
# Host-side entry point
# ---------------------------------------------------------------------------
N_CORES = 8


def prep_inputs(x_shard, weight, bias):
    # lhsT row 18g+c = weight[:, c, g, b]; lhsT col = oc
    wT = np.ascontiguousarray(
        np.transpose(weight, (2, 1, 3, 0)).reshape(54, 3, 64)
    ).astype(np.float32)
    bias2 = np.concatenate([bias, bias]).reshape(128, 1).astype(np.float32)
    return {
        "x": np.ascontiguousarray(x_shard, dtype=np.float32),
        "wT": wT,
        "bias2": bias2,
        "zeros": np.zeros((54, 258), np.float32),
    }


def run(x, weight, bias, trace=False, **build_kwargs):
    from concourse.bass_utils import run_bass_kernel_spmd

    x = np.asarray(x, dtype=np.float32)
    weight = np.asarray(weight, dtype=np.float32)
    bias = np.asarray(bias, dtype=np.float32)

    B = x.shape[0]
    per = B // N_CORES
    nc = build_conv_nc(n_img=per, **build_kwargs)
    _split_excess_waits(nc)
    in_maps = [
        prep_inputs(x[i * per : (i + 1) * per], weight, bias)
        for i in range(N_CORES)
    ]
    res = run_bass_kernel_spmd(nc, in_maps, list(range(N_CORES)), trace=trace)
    y = np.concatenate([res.results[i]["y"] for i in range(N_CORES)], axis=0)
    return y, res


def kernel(x, weight, bias):
    return run(x, weight, bias)[0]
